# revision 1
# baseline (speedup 1.0000x reference)
"""GRU decoder kernel for Trainium2 (Bass/Tile), 8-core data-parallel.

Problem: B=1024, T=2048, V=4, E=16, U=16 Keras-style GRU (reset_after=True,
all activations sigmoid) with embedding lookup fused in.

Key structure exploited:
  * V=4 -> x@kernel+bias0 collapses to a 4-row table; the per-step input
    projection becomes table.T @ onehot_t (a K=4 matmul), prefetchable.
  * Both biases fold into the table (onehot columns sum to 1), except the
    recurrent-h bias which rides in table cols 48:64 (same value all rows).
  * State kept transposed hT [U=16 part, B=128 free] so the recurrent matmul
    needs no per-step transpose.  Gate pre-activations land in two PSUM
    tiles (A: r_pre@0:16,z_pre@32:48; B: hh@0:16,xh@32:48) so ScalarE and
    DVE each read their own tile and every instruction needs at most one
    semaphore wait (TRN2 allows 1 sync wait per instruction).
  * ALL matmul operands sit at partition base 32 -- mixing base partitions
    across matmuls hangs the hardware.
  * h_new = z*h - (z-1)*cand, with (z-1)*cand as one fused
    scalar_tensor_tensor op; per-step 1-element "absorber" ops keep the
    semaphore vector clocks observed so hot-path waits stay at one.
  * Output y_t = h_new transposed to [128b, 16u] via PE (off critical path),
    accumulated 32 steps/psum-bank, 128 steps/SBUF chunk, DMA'd as 1MB blocks.
"""

import os
import numpy as np

import concourse.bass as bass
import concourse.bacc as bacc
import concourse.mybir as mybir
import concourse.tile as tile
from concourse.bass_utils import run_bass_kernel_spmd
from concourse.tile_rust import add_dep_helper

F32 = mybir.dt.float32
B, T, V, E, U = 1024, 2048, 4, 16, 16
NCORES = 8
BC = B // NCORES          # 128 batch rows per core
# Gate rows live at 32-aligned partition offsets (TRN2 requires AP partition
# starts at 0/32/64/96).  Two separate PSUM tiles so each matmul carries at
# most ONE semaphore wait (HW matmul limit):
#   A [48, BC]: r_pre@0:16, z_pre@32:48   (read only by ScalarE sigmoid)
#   B [48, BC]: hh@0:16,    xh@32:48      (read only by DVE v1/v2)
WA = 48
WB = 48

OH_CH = 64                # steps per onehot SBUF chunk
TR_CH = 32                # steps per transpose PSUM bank
OUT_CH = 128              # steps per output SBUF chunk


def build_program(t_steps=T):
    OH_CH = min(globals()["OH_CH"], t_steps)
    TR_CH = min(globals()["TR_CH"], t_steps)
    OUT_CH = min(globals()["OUT_CH"], t_steps)
    assert t_steps % OUT_CH == 0 and OUT_CH % TR_CH == 0
    nc = bacc.Bacc()
    oh_d = nc.declare_dram_parameter("onehot", [V, t_steps * BC], F32, isOutput=False)
    # consts [48, 336]: rows 32:48 cols 0:96 = recF; rows 32:36 cols 96:192 =
    # tableF; rows 32:48 cols 192:208 = identity; rows 32:48 cols 208:336 =
    # h0T.  Everything matmuls touch lives at partition base 32 -- mixing
    # different base partitions across matmuls hangs the hardware.  One
    # tensor -> one DMA -> one semaphore for all constant reads.
    CW = 2 * (WA + WB) + U + BC
    cst_d = nc.declare_dram_parameter("consts", [48, CW], F32, isOutput=False)
    out_d = nc.declare_dram_parameter("out", [BC, t_steps * U], F32, isOutput=True)
    sink_d = nc.dram_tensor("sink", [4, 4], F32)  # tail-absorber scratch

    SIG = mybir.ActivationFunctionType.Sigmoid
    SUB = mybir.AluOpType.subtract
    MULT = mybir.AluOpType.mult

    with tile.TileContext(nc) as tc:
        with (
            tc.tile_pool(name="const", bufs=1) as cpool,
            tc.tile_pool(name="state", bufs=1) as spool,
            tc.tile_pool(name="oh", bufs=3) as ohpool,
            tc.tile_pool(name="work", bufs=3) as wpool,
            tc.tile_pool(name="outb", bufs=2) as opool,
            tc.tile_pool(name="psum", bufs=3, space=bass.MemorySpace.PSUM) as ppool,
            tc.tile_pool(name="trps", bufs=2, space=bass.MemorySpace.PSUM) as trpool,
        ):
            WW = WA + WB
            cst = cpool.tile([48, CW], F32)
            nc.gpsimd.dma_start(cst[:], cst_d[:])
            rec = cst[32:48, 0:WW]
            tab = cst[32:32 + V, WW:2 * WW]
            ident = cst[32:48, 2 * WW:2 * WW + U]
            # h state lives at partition base 32 (rows 32:48) so that
            # SB+SB vector ops pairing it with the z slice of zrz (also at
            # base 32) satisfy the equal-base-partition rule.  Initialized
            # via DVE copy so the DVE observes the consts DMA tick once.
            hTt = spool.tile([48, BC], F32)
            hT = hTt[32:48, :]
            nc.vector.tensor_copy(hT, cst[32:48, 2 * WW + U:CW])
            # Tick-absorber scratch: a 1-column DVE copy of hT after every
            # h update makes the h-writer's DVE tick "observed", so the next
            # step's first h reader on DVE (bb) needs only the ACT tick.
            scr = spool.tile([U, 1], F32)
            nc.vector.tensor_copy(scr[:], hT[:, 0:1])
            # ACT-side absorber scratch: a 1-elem ScalarE copy per step whose
            # self-wait chain keeps all ACT slot-WAW ticks observed, so zrz
            # and cd each carry exactly one real wait.
            sca = spool.tile([1, 1], F32)
            nc.scalar.copy(sca[:], cst[0:1, 0:1])

            # Dummy matmul reading only the consts: absorbs the consts-DMA
            # semaphore wait so the first real matmul carries at most one
            # wait (HW matmul wait-slot limit).
            dps = trpool.tile([U, 8], F32, tag="trps")
            nc.tensor.matmul(dps[:], tab[:, 0:U], tab[:, 0:8],
                             start=True, stop=True)

            oh_sb = None
            out_sb = None
            tr_ps = None
            flush = None  # deferred (copy/dma) emissions, run post-chain
            prev_mmrecA = None
            last_tr = [None]

            def emit_y(i):
                """Transpose y_i = current hT into the output staging path.
                Emitted right after mm_rec(i+1) so the PE does it during the
                chain stall; copies/DMAs are deferred to end of iteration."""
                nonlocal out_sb, tr_ps, flush
                if i % TR_CH == 0:
                    tr_ps = trpool.tile([BC, TR_CH * U], F32, tag="trps")
                if i % OUT_CH == 0:
                    out_sb = opool.tile([BC, OUT_CH * U], F32, tag="outsb")
                k = i % TR_CH
                last_tr[0] = nc.tensor.transpose(
                    tr_ps[:, k * U:(k + 1) * U], hT, ident)
                tr_cur, out_cur = tr_ps, out_sb

                def _flush():
                    if i % TR_CH == TR_CH - 1:
                        q = (i % OUT_CH) // TR_CH
                        nc.vector.tensor_copy(
                            out_cur[:, q * TR_CH * U:(q + 1) * TR_CH * U], tr_cur[:])
                    if i % OUT_CH == OUT_CH - 1:
                        c0 = (i - (OUT_CH - 1)) * U
                        nc.gpsimd.dma_start(out_d[:, c0:c0 + OUT_CH * U], out_cur[:])
                return _flush

            n_chunks = t_steps // OH_CH
            oh_tiles = {}

            def load_oh(c):
                if c >= n_chunks or c in oh_tiles:
                    return
                tl = ohpool.tile([32 + V, OH_CH * BC], F32, tag="oh",
                                 name=f"oh{c}")
                nc.gpsimd.dma_start(
                    tl[32:32 + V, :],
                    oh_d[:, c * OH_CH * BC:(c + 1) * OH_CH * BC])
                oh_tiles[c] = tl

            load_oh(0)
            load_oh(1)
            for t in range(t_steps):
                c = t // OH_CH
                if t % OH_CH == 0:
                    oh_sb = oh_tiles.pop(c)
                    load_oh(c + 2)

                j = t % OH_CH
                oh_t = oh_sb[32:32 + V, j * BC:(j + 1) * BC]
                # Chunk-start steps use a dedicated 1-buf psum slot so their
                # mm_x_B's only unobserved tick is the onehot DMA (psum WAW /
                # WAR ticks are 64 steps old -> elided).
                if j == 0:
                    psB = ppool.tile([WB, BC], F32, tag="stepBx", bufs=1)
                else:
                    psB = ppool.tile([WB, BC], F32, tag="stepB", bufs=2)
                psA = ppool.tile([WA, BC], F32, tag="stepA", bufs=3)
                # input projections (independent of h -> run in PE slack).
                # B first: its psum-WAW self-wait shields A's; A then carries
                # only the zrz WAR tick.
                mmxB = nc.tensor.matmul(psB[:], tab[:, WA:WA + WB], oh_t,
                                        start=True, stop=False)
                if prev_mmrecA is not None:
                    # schedule mm_x_B after the previous mm_rec_A so the DVE
                    # tick it would wait on is already observed
                    add_dep_helper(mmxB.ins, prev_mmrecA.ins, sync=False,
                                   reason="order mmxB after prev mmrecA")
                nc.tensor.matmul(psA[:], tab[:, 0:WA], oh_t,
                                 start=True, stop=False)
                # recurrent projections (critical path); A first -> sigmoid
                # starts as soon as A lands.  tr(t-1) sits between them so
                # its PE tick is covered by v1's wait on mm_rec_B.
                prev_mmrecA = nc.tensor.matmul(psA[:], rec[:, 0:WA], hT,
                                               start=False, stop=True)
                if t >= 1:
                    flush = emit_y(t - 1)
                nc.tensor.matmul(psB[:], rec[:, WA:WA + WB], hT,
                                 start=False, stop=True)

                zrz = wpool.tile([48, BC], F32, tag="zrz")
                nc.scalar.activation(zrz[:], psA[:], SIG)  # r@0:16, z@32:48
                # DVE order: bb, v1, v2, aa, hnew, tick-absorber copy.
                # Keeps every instruction at one semaphore wait (ISA limit):
                # bb waits ACT(zrz) (hnew tick pre-observed via absorber);
                # v1 waits PE only; aa waits ACT(cd); hnew DVE-local.
                v1 = wpool.tile([U, BC], F32, tag="v1")
                nc.vector.tensor_mul(v1[:], zrz[0:U, :], psB[0:U, :])    # r*hh
                v2 = wpool.tile([U, BC], F32, tag="v2")
                nc.vector.tensor_add(v2[:], v1[:], psB[32:48, :])        # +xh
                # bb off the chain head: v1/v2 feed cd sooner
                bb = wpool.tile([48, BC], F32, tag="bb")
                nc.vector.tensor_mul(bb[32:48, :], zrz[32:48, :], hT)    # z*h
                # cand/aa/bb also live at base 32 to pair with z and h
                cd = wpool.tile([48, BC], F32, tag="cd")
                mmcd = nc.scalar.activation(cd[32:48, :], v2[:], SIG)
                aa = wpool.tile([48, BC], F32, tag="aa")
                nc.vector.scalar_tensor_tensor(                          # (z-1)*c
                    aa[32:48, :], zrz[32:48, :], 1.0, cd[32:48, :],
                    op0=SUB, op1=MULT)
                nc.vector.tensor_sub(hT, bb[32:48, :], aa[32:48, :])     # h_new
                if not os.environ.get("K_NO_SCR"):
                    nc.vector.tensor_copy(scr[:], hT[:, 0:1])  # DVE absorber
                if not os.environ.get("K_NO_SCA"):
                    mabs = nc.scalar.copy(sca[:], cst[0:1, 0:1])  # ACT absorber
                    # pin after cd so the self-wait chain stays current
                    add_dep_helper(mabs.ins, mmcd.ins, sync=False,
                                   reason="keep ACT absorber in step order")

                if flush is not None:
                    flush()
                    flush = None

            flush = emit_y(t_steps - 1)
            flush()

            # Kernel-tail sem absorption: the epilogue drain can carry only a
            # few sync waits, so funnel every engine's final tick through SP.
            # ACT absorbs the last PE tick (reads the final transpose psum),
            # then two tiny DMAs absorb the ACT and DVE ticks.
            if not os.environ.get("K_NO_SINK"):
                fps = ppool.tile([U, 8], F32, tag="stepBx", bufs=1)
                mmF = nc.tensor.matmul(fps[:], tab[:, 0:U], tab[:, 0:8],
                                       start=True, stop=True)
                add_dep_helper(mmF.ins, last_tr[0].ins, sync=False,
                               reason="tail absorber runs last on PE")
                sfin = spool.tile([1, 1], F32)
                nc.scalar.copy(sfin[:], fps[0:1, 0:1])
                nc.gpsimd.dma_start(sink_d[0:1, 0:1], sfin[:])

    nc.finalize()
    return nc
    for bb in nc.main_func.blocks:
        insts = bb.instructions
        for di, inst in enumerate(insts):
            if (type(inst).__name__ == "InstDrain"
                    and inst.sync_info and inst.sync_info.on_wait
                    and len(inst.sync_info.on_wait) > 1):
                waits = [w for w in inst.sync_info.on_wait
                         if w.ant_name.startswith(("DMAHW", "DMASW"))]
                nops = []
                for wi, w in enumerate(waits[:-1]):
                    nop = mybir.InstNoOp(name=f"I-drainwait-{bb.name}-{wi}")
                    nop.engine = inst.engine
                    nop.sync_info = mybir.SyncInfo(on_wait=[w], on_update=[])
                    nops.append(nop)
                inst.sync_info = mybir.SyncInfo(
                    on_wait=waits[-1:], on_update=inst.sync_info.on_update)
                bb.instructions = insts[:di] + nops + insts[di:]
                break
    return nc


_PROGRAMS = {}


def _get_program(t_steps):
    if t_steps not in _PROGRAMS:
        _PROGRAMS[t_steps] = build_program(t_steps)
    return _PROGRAMS[t_steps]


def _prep_inputs(inputs, t_steps=T):
    enc = np.ascontiguousarray(np.asarray(inputs["encoder_hidden_state"], dtype=np.float32))
    tg = np.asarray(inputs["targets"])
    emb = np.asarray(inputs["emb"], dtype=np.float32)
    ker = np.asarray(inputs["kernel"], dtype=np.float32)
    rk = np.asarray(inputs["rec_kernel"], dtype=np.float32)
    bias = np.asarray(inputs["bias"], dtype=np.float32)

    table = emb @ ker + bias[0]                     # [4, 48]; cols z|r|h
    tabF = np.zeros((V, WA + WB), np.float32)
    tabF[:, 0:16] = table[:, 16:32] + bias[1][None, 16:32]   # A: r_pre const
    tabF[:, 32:48] = table[:, 0:16] + bias[1][None, 0:16]    # A: z_pre const
    tabF[:, WA + 0:WA + 16] = bias[1][None, 32:48]           # B: hh bias
    tabF[:, WA + 32:WA + 48] = table[:, 32:48]               # B: xh (incl b0h)
    recF = np.zeros((U, WA + WB), np.float32)
    recF[:, 0:16] = rk[:, 16:32]                             # A: r_pre h part
    recF[:, 32:48] = rk[:, 0:16]                             # A: z_pre h part
    recF[:, WA + 0:WA + 16] = rk[:, 32:48]                   # B: hh h part
    WW = WA + WB
    consts = np.zeros((48, 2 * WW + U + BC), np.float32)
    consts[32:48, 0:WW] = recF
    consts[32:32 + V, WW:2 * WW] = tabF
    consts[32:48, 2 * WW:2 * WW + U] = np.eye(U, dtype=np.float32)

    vocab = np.arange(V)
    maps = []
    for k in range(NCORES):
        tg_k = tg[k * BC:(k + 1) * BC, :t_steps]    # [128, t]
        oh = (vocab[:, None, None] == tg_k.T[None, :, :]).astype(np.float32)
        ck = consts.copy()
        ck[32:48, 2 * WW + U:] = enc[k * BC:(k + 1) * BC].T
        maps.append({
            "onehot": np.ascontiguousarray(oh.reshape(V, -1)),
            "consts": ck,
        })
    return maps


def run(inputs, t_steps=T, **run_kwargs):
    nc = _get_program(t_steps)
    maps = _prep_inputs(inputs, t_steps)
    res = run_bass_kernel_spmd(nc, maps, list(range(NCORES)), **run_kwargs)
    outs = [np.asarray(res.results[i]["out"]).reshape(BC, t_steps, U)
            for i in range(NCORES)]
    return np.concatenate(outs, axis=0), res


def kernel(**inputs):
    out, _ = run(inputs)
    return out

